# revision 9
# baseline (speedup 1.0000x reference)
"""Trainium2 Bass kernel for batched dense attention.

Problem: query/key/value [B=8, S=4096, D=128] fp32; out[b,q,d] =
softmax(Q K^T / sqrt(D)) V per batch element.

Sharding: data-parallel over batch. 8 NeuronCores, one batch element per
core; no collectives. Per core, one 4096x4096 attention in layout B
(scores transposed: k on partitions, q on free).

v12 design (ACT-paced; every other engine kept under the exp stream):
  - Per q-group of 512 queries (8 groups), 11 score slabs (10x3kt +
    1x2kt, FD<=1536) -- one fewer ACTIVATE per group than v11 (each
    costs ~293ns of ACT overhead). PSUM: A(3) + B(3) + po(1) + den(1).
    Slab tags alternate by global slab index (parity flips per group).
      mm1 (bf16): S^T[k,q] slab -> PSUM; exp on ScalarE with
      scale=1/sqrt(D), PSUM fp32 -> SBUF bf16 E tiles.
      mm2 (bf16): po[d,q] += V[kt].T @ E[kt], woven at slab-lag 6.
  - Denominator: per-8kt chunk trees on DVE (bf16) -> esum in BF16; the
    128-partition sum+broadcast is ONE bf16 matmul (all-ones stationary)
    into the den bank (v11 used a 2-pass fp32 LOW_HIGH matmul that HOL-
    blocked mm1 at group boundaries). den+recip emitted at si=1 of the
    next group (far from any dependency), epilogue at si=6.
  - Loads: K entirely via PE-transpose (fp32 transposes through the den
    PSUM bank; XBAR reserved for Q), V needs no transpose (DVE cast
    only), Q0 PE-transposed, Q1-Q4 via DVE-cast + XBAR. DMA enqueue
    order matches need-by order: K0 Q0 K1 | K2 V0 K3 K4 Q1 V1 V2 V3 |
    Q2 Q3 Q4.  nat pool bufs=12 so the enqueue ring never WAR-blocks.
  - Last group: mm2(g6) drains 2 slabs/si over si 0-2, epilogue(g6) at
    si=3, mm2(g7) at lag 3 (slabs 0..7 in-stream); denominator tail is
    precombined so after the final exp only 3 chained DVE adds remain
    before the bf16 den matmul -> recip -> mul -> out DMA.
  - Prologue: dummy exp preloads the ACT exp table; 12 junk matmuls warm
    the PE HAM clock gate, interleaved 6/4/2 with the K0/Q0 transposes.
"""

import sys

sys.path.insert(0, "/opt/trn_rl_repo")

import numpy as np

import concourse.bass as bass
import concourse.mybir as mybir
import concourse.tile as tile
from concourse import bacc
from concourse.bass_utils import run_bass_kernel_spmd
from concourse.masks import make_identity

B, S, D = 8, 4096, 128
N_CORES = 8

F32 = mybir.dt.float32
BF16 = mybir.dt.bfloat16

NS = 11           # slabs per 512-query group: 1x2kt + 10x3kt (last grp rev)
MM2_LAG = 6
LAST_G = 7


def slab_info(g, s):
    """(kt0, nkt) for slab s of group g.

    Groups 0-6 put the short (2kt) slab FIRST so that at every group
    boundary the next group's first mm1 (2 matmuls) fits inside the
    previous group's last exp (N=1536); the last group puts it LAST so
    the post-stream denominator tail is short.
    """
    if g == LAST_G:
        return (3 * s, 3) if s < 10 else (30, 2)
    if s == 0:
        return 0, 2
    return 3 * s - 1, 3


def build_attention_core(s=S):
    QG = 512
    N_GROUPS = s // QG
    N_KT = s // 128
    SCALE = 1.0 / np.sqrt(D)

    nc = bacc.Bacc("TRN2", target_bir_lowering=False, debug=False)
    q_d = nc.dram_tensor("q", [s, D], F32, kind="ExternalInput").ap()
    k_d = nc.dram_tensor("k", [s, D], F32, kind="ExternalInput").ap()
    v_d = nc.dram_tensor("v", [s, D], F32, kind="ExternalInput").ap()
    # output is O^T [D, s]; host transposes
    o_d = nc.dram_tensor("out", [D, s], F32, kind="ExternalOutput").ap()

    with tile.TileContext(nc) as tc:
        with (
            tc.tile_pool(name="persist", bufs=1) as persist,
            tc.tile_pool(name="loads", bufs=3) as loads,
            tc.tile_pool(name="ebuf", bufs=2) as ebuf,
            tc.tile_pool(name="tree", bufs=1) as treep,
            tc.tile_pool(name="small", bufs=2) as small,
            tc.tile_pool(name="ps", bufs=1, space="PSUM") as ps,
        ):
            ktb = persist.tile([128, N_KT, 128], BF16)   # K^T [d, kt, k]
            qtb = persist.tile([128, N_KT, 128], BF16)   # Q^T [d, qt, q]
            vtb = persist.tile([128, N_KT, 128], BF16)   # V   [k, kt, d]
            ones = persist.tile([128, 128], BF16)
            nc.vector.memset(ones[:], 1.0)
            wz = persist.tile([128, 128], BF16)          # warmup zeros
            nc.vector.memset(wz[:], 0.0)
            dumm = persist.tile([128, 8], F32)
            nc.vector.memset(dumm[:], 0.0)
            bias0 = persist.tile([128, 1], F32)
            nc.vector.memset(bias0[:], 0.0)
            ident = persist.tile([128, 128], F32)
            make_identity(nc, ident[:])
            # tree scratch: [0:4] t4, [4:6] t2, [6+j] C_j, [10] H1, [11] H2
            T = treep.tile([128, 12, QG], BF16, name="tree")

            # ACT exp-table preload while loads run
            nc.scalar.activation(dumm[:], dumm[:],
                                 mybir.ActivationFunctionType.Exp,
                                 bias=bias0[:], scale=1.0)

            def warm(n):
                wps = ps.tile([128, 512], F32, tag="po", name="wps")
                for _ in range(n):
                    nc.tensor.matmul(wps[:, :128], wz[:], wz[:],
                                     start=True, stop=True)

            nat_slots = {}

            def emit_nat(src_d, r0, nrows):
                """sync DMA fp32 rows [r0, r0+nrows) into a nat slot."""
                nt = nrows // 128
                nat = loads.tile([128, 8, 128], F32, tag="nat", name="nat",
                                 bufs=10)
                nc.sync.dma_start(
                    nat[:, :nt, :],
                    src_d[r0:r0 + nrows, :].rearrange(
                        "(t p) d -> p t d", p=128))
                nat_slots[(src_d.name, r0)] = nat

            def emit_ct(src_d, r0, nrows, dst, eng=None):
                """DVE cast to bf16 + XBAR transpose into dst."""
                nt = nrows // 128
                t0 = r0 // 128
                nat = nat_slots.pop((src_d.name, r0))
                natb = loads.tile([128, 8, 128], BF16, tag="natb",
                                  name="natb", bufs=3)
                nc.vector.tensor_copy(natb[:, :nt, :], nat[:, :nt, :])
                (eng or nc.sync).dma_start_transpose(
                    dst[:, t0:t0 + nt, :],
                    natb[:, :nt, :].rearrange("p t d -> p (t d)"))

            def emit_pt(src_d, r0, nrows, dst):
                """PE-transpose path: fp32 transposes through the den-tag
                PSUM bank, DVE copy-cast into dst."""
                nt = nrows // 128
                t0 = r0 // 128
                nat = nat_slots.pop((src_d.name, r0))
                for b0 in range(0, nt, 4):
                    nb = min(4, nt - b0)
                    ptr = ps.tile([128, 4, 128], F32, tag="den", name="ptr")
                    for i in range(nb):
                        nc.tensor.transpose(ptr[:, i, :], nat[:, b0 + i, :],
                                            ident[:])
                    nc.vector.tensor_copy(
                        dst[:, t0 + b0:t0 + b0 + nb, :], ptr[:, :nb, :])

            def emit_vload(r0, nrows):
                """SWDGE (gpsimd ring) DMA with in-flight fp32->bf16 cast,
                straight into vtb -- no nat staging, no DVE cast, and a
                second descriptor ring running in parallel with sync."""
                nt = nrows // 128
                t0 = r0 // 128
                nc.gpsimd.dma_start(
                    vtb[:, t0:t0 + nt, :],
                    v_d[r0:r0 + nrows, :].rearrange(
                        "(t p) d -> p t d", p=128))

            gate = persist.tile([128, 8], BF16)

            def emit_vgate():
                """tiny gpsimd read of ktb[:, 15] -- stalls the gpsimd
                queue (and therefore the V-load SWDGE enqueues behind it)
                until K's first 2048 rows are transposed, so the V stream
                doesn't steal HBM bandwidth from the K critical path."""
                nc.gpsimd.tensor_copy(gate[:], ktb[:, 15, :8])
                emit_vload(0, 1024)
                emit_vload(1024, 1024)
                emit_vload(2048, 1024)
                emit_vload(3072, 1024)

            # prologue: K/Q on the sync (HWDGE) ring in need-order; V on the
            # gpsimd (SWDGE) ring behind the gate. K0/Q0 PE-transposed with
            # HAM-warmup matmuls interleaved so the PE clock gate is
            # released by the time the mm1 stream starts.
            emit_nat(k_d, 0, 384)      # K0  (sync ring)
            emit_nat(q_d, 0, 512)      # Q0
            emit_nat(k_d, 384, 640)    # K1
            warm(6)
            emit_pt(k_d, 0, 384, ktb)
            warm(2)
            emit_pt(q_d, 0, 512, qtb)
            warm(3)

            load_sched = {
                (0, 0): [("nat", k_d, 1024, 1024), ("pt", k_d, 384, 640, ktb)],
                (0, 1): [("pt", k_d, 1024, 1024, ktb)],
                (0, 2): [("nat", k_d, 2048, 1024), ("vgate",)],
                (0, 3): [("nat", k_d, 3072, 1024), ("nat", q_d, 512, 512)],
                (0, 4): [("pt", k_d, 2048, 1024, ktb)],
                (0, 6): [("pt", k_d, 3072, 1024, ktb), ("ct", q_d, 512, 512, qtb)],
                (1, 0): [("nat", q_d, 1024, 1024)],
                (1, 2): [("ct", q_d, 1024, 1024, qtb)],
                (1, 6): [("nat", q_d, 2048, 1024)],
                (1, 8): [("ct", q_d, 2048, 1024, qtb)],
                (2, 6): [("nat", q_d, 3072, 1024)],
                (2, 8): [("ct", q_d, 3072, 1024, qtb)],
            }

            def run_load_step(step):
                if step[0] == "nat":
                    emit_nat(step[1], step[2], step[3])
                elif step[0] == "ct":
                    emit_ct(step[1], step[2], step[3], step[4])
                elif step[0] == "vgate":
                    emit_vgate()
                else:
                    emit_pt(step[1], step[2], step[3], step[4])

            e_tiles = [None] * N_GROUPS
            po_tiles = [None] * N_GROUPS
            esums = [None] * N_GROUPS
            rdens = [None] * N_GROUPS

            def slab_tag(g, si):
                return "A" if (g * NS + si) % 2 == 0 else "B"

            def emit_mm1(g, si):
                kt0, nkt = slab_info(g, si)
                tag = slab_tag(g, si)
                psl = ps.tile([128, nkt * QG], F32, tag=tag,
                              name="ps_%s" % tag, padded_shape=[128, 3 * QG])
                qv = qtb[:, 4 * g:4 * g + 4, :].rearrange("p a b -> p (a b)")
                for i in range(nkt):
                    nc.tensor.matmul(psl[:, i * QG:(i + 1) * QG],
                                     ktb[:, kt0 + i, :], qv,
                                     start=True, stop=True)
                return psl

            def emit_exp(g, si, psl):
                kt0, nkt = slab_info(g, si)
                nc.scalar.activation(
                    e_tiles[g][:, kt0:kt0 + nkt, :].rearrange(
                        "p a b -> p (a b)"),
                    psl[:],
                    mybir.ActivationFunctionType.Exp,
                    bias=bias0[:], scale=float(SCALE))

            def emit_mm2(g, si, first=False):
                """mm2 slab; `first` marks the first-EXECUTED slab of the
                group's po accumulation (slabs run out of kt order, so the
                start flag follows emission order, not kt==0)."""
                kt0, nkt = slab_info(g, si)
                if first:
                    po_tiles[g] = ps.tile([128, QG], F32, tag="po", name="po")
                for i in range(nkt):
                    kt = kt0 + i
                    nc.tensor.matmul(
                        po_tiles[g][:], vtb[:, kt, :], e_tiles[g][:, kt, :],
                        start=(first and i == 0), stop=(kt == N_KT - 1),
                        skip_group_check=True)

            def emit_chunk_tree(g, j):
                """8-kt chunk j -> C_j = T[:, 6+j] (bf16)."""
                e = e_tiles[g]
                o = 8 * j
                nc.vector.tensor_add(
                    T[:, 0:4, :], e[:, o:o + 8:2, :], e[:, o + 1:o + 8:2, :])
                nc.vector.tensor_add(
                    T[:, 4:6, :], T[:, 0:4:2, :], T[:, 1:4:2, :])
                nc.vector.tensor_add(T[:, 6 + j, :], T[:, 4, :], T[:, 5, :])

            def emit_h1(g):
                nc.vector.tensor_add(T[:, 10, :], T[:, 6, :], T[:, 7, :])

            def emit_esum(g):
                """group-end combine: esum (bf16) = C0+C1+C2+C3."""
                nc.vector.tensor_add(T[:, 11, :], T[:, 8, :], T[:, 9, :])
                esum = small.tile([128, QG], BF16, tag="esum")
                nc.vector.tensor_add(esum[:], T[:, 10, :], T[:, 11, :])
                esums[g] = esum

            def emit_den(g):
                """128-partition sum with broadcast: one bf16 matmul
                (all-ones stationary) into the den bank, then recip."""
                den_ps = ps.tile([128, QG], F32, tag="den", name="den_ps")
                nc.tensor.matmul(den_ps[:], ones[:], esums[g][:],
                                 start=True, stop=True)
                rden = small.tile([128, QG], F32, tag="rden")
                nc.vector.reciprocal_approx_fast(rden[:], den_ps[:])
                rdens[g] = rden

            def emit_epilogue(g):
                ob = small.tile([128, QG], F32, tag="ob")
                nc.vector.tensor_mul(ob[:], po_tiles[g][:], rdens[g][:])
                nc.sync.dma_start(o_d[:, g * QG:(g + 1) * QG], ob[:])

            for g in range(N_GROUPS):
                e_tiles[g] = ebuf.tile([128, N_KT, QG], BF16, tag="E",
                                       name="e_g")
                last = g == N_GROUPS - 1

                def emit_pair(dst_slot, kt):
                    nc.vector.tensor_add(
                        T[:, dst_slot, :], e_tiles[g][:, kt, :],
                        e_tiles[g][:, kt + 1, :])

                for si in range(NS):
                    psl = emit_mm1(g, si)
                    emit_exp(g, si, psl)
                    # mm2 weave: exactly one mm2 slab per exp window, with
                    # the short (2kt) mm2 slab paired into the short (2kt)
                    # exp window at si=0.
                    if last:
                        # drain mm2(g-1) 2 slabs/si over si 0-2, then run
                        # mm2(g) at lag 3 so only slabs 8-10 remain after
                        # the final exp.
                        if si == 0:
                            emit_mm2(g - 1, 0)
                            emit_mm2(g - 1, 5)
                        elif si == 1:
                            emit_mm2(g - 1, 6)
                            emit_mm2(g - 1, 7)
                        elif si == 2:
                            emit_mm2(g - 1, 8)
                            emit_mm2(g - 1, 9)
                            emit_den(g - 1)
                        elif si == 3:
                            emit_mm2(g - 1, 10)
                            emit_epilogue(g - 1)
                            emit_mm2(g, 0, first=True)
                        else:
                            emit_mm2(g, si - 3)
                    else:
                        if si == 0 and g > 0:
                            emit_mm2(g - 1, 0)
                        elif 1 <= si <= 6 and g > 0:
                            emit_mm2(g - 1, 4 + si)
                            if si == 1:
                                emit_den(g - 1)
                        elif si == 7:
                            if g > 0:
                                emit_epilogue(g - 1)
                            emit_mm2(g, 1, first=True)
                        elif si >= 8:
                            emit_mm2(g, si - 6)
                    if si == 3:
                        emit_chunk_tree(g, 0)
                    elif si == 6:
                        emit_chunk_tree(g, 1)
                    elif si == 7:
                        emit_h1(g)
                    elif si == 8:
                        emit_chunk_tree(g, 2)
                    elif last and si == 9:
                        # precombine for the tail: pairs of kt 24-27 and
                        # H = C0+C1+C2 (via h1 + C2)
                        emit_pair(0, 24)
                        emit_pair(1, 26)
                        nc.vector.tensor_add(T[:, 11, :], T[:, 10, :],
                                             T[:, 8, :])
                    elif last and si == 10:
                        emit_pair(2, 28)
                        nc.vector.tensor_add(T[:, 3, :], T[:, 0, :],
                                             T[:, 1, :])
                        nc.vector.tensor_add(T[:, 4, :], T[:, 11, :],
                                             T[:, 3, :])
                    for step in load_sched.get((g, si), ()):
                        run_load_step(step)
                if not last:
                    emit_chunk_tree(g, 3)
                    emit_esum(g)

            # drain (last group g=7): tail after final exp is 3 chained DVE
            # adds -> bf16 den matmul -> recip -> mul -> out DMA, with the
            # remaining mm2 slabs woven around the den matmul on the PE.
            g = N_GROUPS - 1
            emit_pair(5, 30)                                   # E30+E31
            nc.vector.tensor_add(T[:, 6, :], T[:, 2, :], T[:, 5, :])
            esum = small.tile([128, QG], BF16, tag="esum")
            nc.vector.tensor_add(esum[:], T[:, 4, :], T[:, 6, :])
            esums[g] = esum
            emit_mm2(g, 8)
            emit_mm2(g, 9)
            emit_den(g)
            emit_mm2(g, 10)
            emit_epilogue(g)

    nc.compile()
    return nc


_NC_CACHE = None


def kernel(query: np.ndarray, key: np.ndarray, value: np.ndarray) -> np.ndarray:
    global _NC_CACHE
    if _NC_CACHE is None:
        _NC_CACHE = build_attention_core()
    nc = _NC_CACHE
    in_maps = [
        {
            "q": np.ascontiguousarray(query[i]),
            "k": np.ascontiguousarray(key[i]),
            "v": np.ascontiguousarray(value[i]),
        }
        for i in range(N_CORES)
    ]
    res = run_bass_kernel_spmd(nc, in_maps, core_ids=list(range(N_CORES)))
    # per-core output is O^T [D, s]; transpose back
    return np.stack(
        [np.ascontiguousarray(res.results[i]["out"].T)
         for i in range(N_CORES)], axis=0)


if __name__ == "__main__":
    rng = np.random.default_rng(0)
    q = rng.standard_normal((B, S, D), dtype=np.float32)
    k = rng.standard_normal((B, S, D), dtype=np.float32)
    v = rng.standard_normal((B, S, D), dtype=np.float32)
    out = kernel(q, k, v)
    print(out.shape, out.dtype)
